# revision 1
# baseline (speedup 1.0000x reference)
"""KMeansProbSampler Trainium2 kernel (8-core SPMD).

Algorithm (per reference): 8 iterations of
  d2[p,c]   = (h_p - a_c)^2 + (w_p - b_c)^2        (pixel grid 1024x1024, C=128)
  assign[p] = argmin_c max(1, sqrt(d2))            (first-index tie break)
  new[c]    = sum_{p: assign==c} coords_p * heatmap_p / max(1, sqrt(min d2))

Mapping:
  - Shard pixel rows across 8 cores (128 rows each). A "tile" is one image
    column within the shard: 128 pixels on SBUF partitions.
  - d2 for a tile x all 128 clusters via one K=4 PE matmul using
    block-recentered coordinates: lhsT rows [h', w', 1, h'^2+w'^2] (host
    precomputed, streamed from DRAM per 128-column block), rhs rows
    [-2a', -2b', a'^2+b'^2 (+dup mask), 1] built on device each iteration.
    Recentring (h-512, w - block_center) keeps the expanded form's
    cancellation error small at small d2.
  - argmin is replaced by value-matching: m2 = min_c d2 (DVE segmented
    reduce), scaled-one-hot = (d2 == m2) * (1/max(1,sqrt(m2))) in a single
    DVE tensor_scalar op. Duplicate clusters (empty clusters collapse to
    (0,0) from iteration 2 on) get +1e30 in the rhs norm row, computed on
    device, so the first duplicate wins exactly like jnp.argmin.
  - scatter: PE matmul acc[c, 0:2] += soh^T @ [h*hm, w*hm] (host precomputed
    moving operand, N=2), PSUM-accumulated over all 1024 tiles.
  - per-iteration AllReduce of the [128, 2] partial sums across 8 cores.
"""

import os
import sys

import numpy as np

H = 1024
W = 1024
C = 128
N_ITER = 8
NCORES = 8
RPC = H // NCORES  # rows per core
P = 128            # partitions = pixels per tile
NT = W             # tiles (columns) per core
TPB = 128          # tiles per w-block
WG = 4             # tiles per PSUM group ([128, 512] = one bank)
GPB = 4            # groups per sqrt/recip batch (16 tiles)
S_H = 512.0        # global h recentering
BIG = 1.0e30       # duplicate-cluster mask

_REPO_CANDIDATES = ("/opt/trn_rl_repo", "/root/.axon_site/_ro/trn_rl_repo")


def _ensure_repo():
    try:
        import concourse  # noqa: F401
        return
    except ImportError:
        pass
    for p in _REPO_CANDIDATES:
        if os.path.isdir(p):
            sys.path.insert(0, p)
            break
    import concourse  # noqa: F401


def build_nc(n_iter: int = N_ITER, nt: int = NT, ncores: int = NCORES):
    """Build the SPMD Bass program (same program for every core)."""
    _ensure_repo()
    import concourse.bacc as bacc
    import concourse.mybir as mybir
    import concourse.tile as tile

    f32 = mybir.dt.float32
    Alu = mybir.AluOpType
    Act = mybir.ActivationFunctionType
    X = mybir.AxisListType.X

    nblk = (nt + TPB - 1) // TPB
    assert nt % TPB == 0

    nc = bacc.Bacc(
        "TRN2",
        target_bir_lowering=False,
        debug=False,
        num_devices=ncores,
    )

    # ---- I/O ----
    pixT_d = nc.dram_tensor("pixT", [4 * nblk, TPB * P], f32, kind="ExternalInput")
    vhw_d = nc.dram_tensor("vhw", [P, 2 * nt], f32, kind="ExternalInput")
    ext0_d = nc.dram_tensor("ext0", [4 * nblk, C], f32, kind="ExternalInput")
    ident_d = nc.dram_tensor("ident", [P, P], f32, kind="ExternalInput")
    ltri_d = nc.dram_tensor("ltri", [P, P], f32, kind="ExternalInput")
    out_d = nc.dram_tensor("out", [C, 2], f32, kind="ExternalOutput")

    with tile.TileContext(nc) as tc:
        from contextlib import ExitStack

        with ExitStack() as st:
            const = st.enter_context(tc.tile_pool(name="const", bufs=1))
            stpool = st.enter_context(tc.tile_pool(name="stage", bufs=2))
            spool = st.enter_context(tc.tile_pool(name="s", bufs=10))
            mpool = st.enter_context(tc.tile_pool(name="m2", bufs=3))
            sohp = st.enter_context(tc.tile_pool(name="soh", bufs=8))
            smal = st.enter_context(tc.tile_pool(name="small", bufs=4))
            eqp = st.enter_context(tc.tile_pool(name="eq", bufs=2))
            psd = st.enter_context(tc.tile_pool(name="psd", bufs=3, space="PSUM"))
            psa = st.enter_context(tc.tile_pool(name="psa", bufs=1, space="PSUM"))
            pse = st.enter_context(tc.tile_pool(name="pse", bufs=2, space="PSUM"))
            dram = st.enter_context(tc.tile_pool(name="dram", bufs=2, space="DRAM"))

            # ---- persistent SBUF state ----
            vhw = const.tile([P, 2 * nt], f32)
            ident = const.tile([P, P], f32)
            ltri = const.tile([P, P], f32)
            exts = [const.tile([4, C], f32, name=f"ext{b}", tag=f"ext{b}")
                    for b in range(nblk)]

            nc.gpsimd.dma_start(vhw[:], vhw_d[:])
            nc.gpsimd.dma_start(ident[:], ident_d[:])
            nc.gpsimd.dma_start(ltri[:], ltri_d[:])
            for b in range(nblk):
                nc.gpsimd.dma_start(exts[b][:], ext0_d[4 * b:4 * b + 4, :])

            arout_prev = None
            for it in range(n_iter):
                acc = psa.tile([C, 2], f32, space="PSUM")
                for b in range(nblk):
                    stage = stpool.tile([4, TPB * P], f32, tag="stage")
                    nc.gpsimd.dma_start(stage[:], pixT_d[4 * b:4 * b + 4, :])
                    for gl in range(TPB // WG):
                        t0 = b * TPB + gl * WG
                        psum_d = psd.tile([P, WG * P], f32, space="PSUM")
                        for tau in range(WG):
                            loc = gl * WG + tau
                            nc.tensor.matmul(
                                out=psum_d[:, tau * P:(tau + 1) * P],
                                lhsT=stage[:, loc * P:(loc + 1) * P],
                                rhs=exts[b][:],
                                start=(tau == 0),
                                stop=(tau == WG - 1),
                            )
                        s = spool.tile([P, WG * P], f32)
                        nc.scalar.copy(out=s[:], in_=psum_d[:])

                        gg = gl % GPB
                        if gg == 0:
                            m2 = mpool.tile([P, GPB * WG], f32, tag="m2")
                            batch_s = []
                            batch_t0 = t0
                        batch_s.append(s)
                        nc.vector.tensor_reduce(
                            out=m2[:, gg * WG:(gg + 1) * WG],
                            in_=s[:].rearrange("p (n x) -> p n x", x=P),
                            axis=X,
                            op=Alu.min,
                        )
                        if gg == GPB - 1:
                            # batched 1/max(1, sqrt(m2)) for these 16 tiles
                            rec = mpool.tile([P, GPB * WG], f32, tag="rec")
                            sq = mpool.tile([P, GPB * WG], f32, tag="sq")
                            nc.vector.tensor_scalar(
                                out=sq[:], in0=m2[:], scalar1=1.0, scalar2=None,
                                op0=Alu.max,
                            )
                            nc.scalar.activation(out=sq[:], in_=sq[:],
                                                 func=Act.Sqrt)
                            nc.vector.reciprocal(out=rec[:], in_=sq[:])
                            for q, s_q in enumerate(batch_s):
                                for tau in range(WG):
                                    t = batch_t0 + q * WG + tau
                                    col = q * WG + tau
                                    soh = sohp.tile([P, P], f32)
                                    nc.vector.tensor_scalar(
                                        out=soh[:],
                                        in0=s_q[:, tau * P:(tau + 1) * P],
                                        scalar1=m2[:, col:col + 1],
                                        scalar2=rec[:, col:col + 1],
                                        op0=Alu.is_equal,
                                        op1=Alu.mult,
                                    )
                                    nc.tensor.matmul(
                                        out=acc[:],
                                        lhsT=soh[:],
                                        rhs=vhw[:, 2 * t:2 * t + 2],
                                        start=(t == 0),
                                        stop=(t == nt - 1),
                                    )

                # ---- partial [C,2] -> AllReduce ----
                part = smal.tile([C, 2], f32, tag="part")
                nc.scalar.copy(out=part[:], in_=acc[:])
                arin = dram.tile([C, 2], f32)
                arout = dram.tile([C, 2], f32)
                nc.gpsimd.dma_start(arin[:], part[:])
                nc.gpsimd.collective_compute(
                    "AllReduce",
                    Alu.add,
                    replica_groups=[list(range(ncores))],
                    ins=[arin[:].opt()],
                    outs=[arout[:].opt()],
                )
                arout_prev = arout

                if it == n_iter - 1:
                    break

                # ---- epilogue: rebuild per-block ext from reduced clusters ----
                ncs = smal.tile([C, 2], f32, tag="ncs")
                nc.gpsimd.dma_start(ncs[:], arout[:])

                # broadcast a and b along free dim: bc[i, j] = coord_j
                abc = pse.tile([C, C], f32, space="PSUM", tag="bc")
                nc.tensor.transpose(
                    out=abc[:], in_=ncs[:, 0:1].to_broadcast([C, C]),
                    identity=ident[:],
                )
                eqa = eqp.tile([C, C], f32, tag="eqa")
                nc.vector.tensor_scalar(
                    out=eqa[:], in0=abc[:], scalar1=ncs[:, 0:1], scalar2=None,
                    op0=Alu.is_equal,
                )
                bbc = pse.tile([C, C], f32, space="PSUM", tag="bc")
                nc.tensor.transpose(
                    out=bbc[:], in_=ncs[:, 1:2].to_broadcast([C, C]),
                    identity=ident[:],
                )
                eqb = eqp.tile([C, C], f32, tag="eqb")
                nc.vector.tensor_scalar(
                    out=eqb[:], in0=bbc[:], scalar1=ncs[:, 1:2], scalar2=None,
                    op0=Alu.is_equal,
                )
                nc.vector.tensor_tensor(out=eqa[:], in0=eqa[:], in1=eqb[:],
                                        op=Alu.mult)
                nc.vector.tensor_tensor(out=eqa[:], in0=eqa[:], in1=ltri[:],
                                        op=Alu.mult)
                # cf[i] = count of earlier duplicates of cluster i
                cfs = smal.tile([C, 1], f32, tag="cfs")
                nc.vector.tensor_reduce(out=cfs[:], in_=eqa[:], axis=X,
                                        op=Alu.add)
                nc.vector.tensor_scalar(
                    out=cfs[:], in0=cfs[:], scalar1=BIG, scalar2=None,
                    op0=Alu.mult,
                )
                apc = smal.tile([C, 1], f32, tag="apc")
                nc.vector.tensor_scalar(out=apc[:], in0=ncs[:, 0:1],
                                        scalar1=S_H, scalar2=None,
                                        op0=Alu.subtract)
                basec = smal.tile([C, 1], f32, tag="basec")
                nc.vector.tensor_tensor(out=basec[:], in0=apc[:], in1=apc[:],
                                        op=Alu.mult)
                nc.vector.tensor_tensor(out=basec[:], in0=basec[:], in1=cfs[:],
                                        op=Alu.add)
                bpc = smal.tile([C, 1], f32, tag="bpc")
                b2c = smal.tile([C, 1], f32, tag="b2c")
                for b in range(nblk):
                    w0 = float(b * TPB + 64)
                    extt = eqp.tile([C, 4], f32, tag="extt")
                    nc.vector.tensor_scalar(
                        out=extt[:, 0:1], in0=ncs[:, 0:1], scalar1=S_H,
                        scalar2=-2.0, op0=Alu.subtract, op1=Alu.mult,
                    )
                    nc.vector.tensor_scalar(
                        out=extt[:, 1:2], in0=ncs[:, 1:2], scalar1=w0,
                        scalar2=-2.0, op0=Alu.subtract, op1=Alu.mult,
                    )
                    nc.vector.tensor_scalar(
                        out=bpc[:], in0=ncs[:, 1:2], scalar1=w0, scalar2=None,
                        op0=Alu.subtract,
                    )
                    nc.vector.tensor_tensor(out=b2c[:], in0=bpc[:], in1=bpc[:],
                                            op=Alu.mult)
                    nc.vector.tensor_tensor(out=extt[:, 2:3], in0=b2c[:],
                                            in1=basec[:], op=Alu.add)
                    nc.vector.memset(extt[:, 3:4], 1.0)
                    extp = pse.tile([4, C], f32, space="PSUM", tag="extp")
                    nc.tensor.transpose(out=extp[:], in_=extt[:],
                                        identity=ident[:])
                    nc.scalar.copy(out=exts[b][:], in_=extp[:])

            # final output
            nc.gpsimd.dma_start(out_d[:], arout_prev[:])

    nc.compile()
    return nc


def make_core_inputs(core: int, clusters: np.ndarray, heatmap: np.ndarray,
                     nt: int = NT):
    """Host-precomputed per-core constant tables."""
    nblk = nt // TPB
    r0 = core * RPC
    hs = (np.arange(P, dtype=np.float32) + np.float32(r0))
    hp = (hs - np.float32(S_H)).astype(np.float32)

    pixT = np.zeros((4 * nblk, TPB * P), np.float32)
    for b in range(nblk):
        w0 = np.float32(b * TPB + 64)
        for tau in range(TPB):
            t = b * TPB + tau
            wp = np.float32(np.float32(t) - w0)
            sl = slice(tau * P, (tau + 1) * P)
            pixT[4 * b + 0, sl] = hp
            pixT[4 * b + 1, sl] = wp
            pixT[4 * b + 2, sl] = 1.0
            pixT[4 * b + 3, sl] = (hp * hp + wp * wp).astype(np.float32)

    hm = heatmap[r0:r0 + RPC, :nt].astype(np.float32)
    vhw = np.empty((P, 2 * nt), np.float32)
    vhw[:, 0::2] = (hs[:, None] * hm).astype(np.float32)
    vhw[:, 1::2] = (np.arange(nt, dtype=np.float32)[None, :] * hm).astype(np.float32)

    a = clusters[:, 0].astype(np.float32)
    b_ = clusters[:, 1].astype(np.float32)
    ext0 = np.zeros((4 * nblk, C), np.float32)
    ap = (a - np.float32(S_H)).astype(np.float32)
    for b in range(nblk):
        w0 = np.float32(b * TPB + 64)
        bp = (b_ - w0).astype(np.float32)
        ext0[4 * b + 0] = (np.float32(-2.0) * ap).astype(np.float32)
        ext0[4 * b + 1] = (np.float32(-2.0) * bp).astype(np.float32)
        ext0[4 * b + 2] = (ap * ap + bp * bp).astype(np.float32)
        ext0[4 * b + 3] = 1.0

    return {
        "pixT": pixT,
        "vhw": vhw,
        "ext0": ext0,
        "ident": np.eye(P, dtype=np.float32),
        "ltri": np.tril(np.ones((P, P), np.float32), -1),
    }


_NC_CACHE = {}


def kernel(clusters: np.ndarray, heatmap: np.ndarray) -> np.ndarray:
    _ensure_repo()
    from concourse.bass_utils import run_bass_kernel_spmd

    clusters = np.asarray(clusters, np.float32)
    heatmap = np.asarray(heatmap, np.float32)

    key = (N_ITER, NT)
    if key not in _NC_CACHE:
        _NC_CACHE[key] = build_nc()
    nc = _NC_CACHE[key]

    in_maps = [make_core_inputs(k, clusters, heatmap) for k in range(NCORES)]
    res = run_bass_kernel_spmd(nc, in_maps, list(range(NCORES)))
    return np.asarray(res.results[0]["out"], np.float32)


if __name__ == "__main__":
    _ensure_repo()
    nc = build_nc(n_iter=int(sys.argv[1]) if len(sys.argv) > 1 else 1,
                  nt=int(sys.argv[2]) if len(sys.argv) > 2 else 128)
    print("built + compiled OK")

